# revision 14
# baseline (speedup 1.0000x reference)
import os
import sys

sys.path.insert(0, "/opt/trn_rl_repo")
import numpy as np

N, M, D, C = 4096, 8192, 1024, 128
NCORES = 8
NL = N // NCORES  # 512 query rows per core
NJ = M // 128  # 64 xn chunks
ND = D // 128  # 8 contraction chunks
G = 16  # xn chunks per activation phase group
NG = NJ // G
SHIFT = 45.0  # ~mean distance; exp(SHIFT - dist) stays in fp16 range

_CACHED_NC = None
LAST_RESULT = None


def _build_nc():
    import concourse.bacc as bacc
    import concourse.mybir as mybir
    import concourse.tile as tile
    import concourse.bass as bass

    f32 = mybir.dt.float32
    f16 = mybir.dt.float16
    AF = mybir.ActivationFunctionType

    nc = bacc.Bacc(target_bir_lowering=False)
    xnT2_h = nc.declare_dram_parameter("xnT2", [NJ, 128, D], f16, isOutput=False)
    xT_h = nc.declare_dram_parameter("xT", [128, ND, NL], f16, isOutput=False)
    yt_h = nc.declare_dram_parameter("yt", [128, NJ, C], f16, isOutput=False)
    xnsq_h = nc.declare_dram_parameter("xnsq", [128, NJ], f32, isOutput=False)
    xsq_h = nc.declare_dram_parameter("xsq", [1, NL], f32, isOutput=False)
    ones_h = nc.declare_dram_parameter("ones", [1, 128], f32, isOutput=False)
    shift_h = nc.declare_dram_parameter("shiftv", [128, 1], f32, isOutput=False)
    out_u_h = nc.declare_dram_parameter("out_u", [C, NL], f32, isOutput=True)
    out_es_h = nc.declare_dram_parameter("out_es", [128, NL], f32, isOutput=True)

    with tile.TileContext(nc) as tc:
        with (
            tc.tile_pool(name="const", bufs=1) as cpool,
            tc.tile_pool(name="dgrp", bufs=2) as dpool,
            tc.tile_pool(name="egrp", bufs=2) as epool,
            tc.tile_pool(name="stream", bufs=3) as spool,
            tc.tile_pool(name="scps", bufs=2, space=bass.MemorySpace.PSUM) as ppool,
            tc.tile_pool(name="acps", bufs=1, space=bass.MemorySpace.PSUM) as upool,
        ):
            xT_sb = cpool.tile([128, ND, NL], f16)
            y_sb = cpool.tile([128, NJ, C], f16)
            xnsq_sb = cpool.tile([128, NJ], f32)
            xsq_sb = cpool.tile([1, NL], f32)
            ones_sb = cpool.tile([1, 128], f32)
            shift_sb = cpool.tile([128, 1], f32)
            xsq_bc = cpool.tile([128, NL], f32)
            esum = cpool.tile([128, NL], f32)
            out_sb = cpool.tile([C, NL], f32)

            nc.sync.dma_start(out=xT_sb, in_=xT_h[:])
            nc.sync.dma_start(out=y_sb, in_=yt_h[:])
            nc.sync.dma_start(out=xnsq_sb, in_=xnsq_h[:])
            nc.sync.dma_start(out=xsq_sb, in_=xsq_h[:])
            nc.sync.dma_start(out=ones_sb, in_=ones_h[:])
            nc.sync.dma_start(out=shift_sb, in_=shift_h[:])

            # broadcast ||x||^2 row to all 128 partitions via K=1 matmul
            xsq_ps = ppool.tile([128, NL], f32)
            nc.tensor.matmul(xsq_ps, ones_sb, xsq_sb, start=True, stop=True)
            nc.vector.tensor_copy(out=xsq_bc, in_=xsq_ps)

            upsum = upool.tile([C, NL], f32)

            def upsum_block(pg, pebuf):
                for jl in range(G):
                    pj = pg * G + jl
                    nc.tensor.matmul(
                        upsum,
                        y_sb[:, pj, :],
                        pebuf[:, jl, :],
                        start=(pj == 0),
                        stop=(pj == NJ - 1),
                    )
                    if pj == 0:
                        nc.vector.tensor_copy(out=esum, in_=pebuf[:, jl, :])
                    else:
                        nc.vector.tensor_add(out=esum, in0=esum, in1=pebuf[:, jl, :])

            prev = None
            for g in range(NG):
                dbuf = dpool.tile([128, G, NL], f32)
                ebuf = epool.tile([128, G, NL], f16)
                for jl in range(G):
                    j = g * G + jl
                    xn_t = spool.tile([128, D], f16)
                    nc.sync.dma_start(out=xn_t, in_=xnT2_h[j])
                    scores = ppool.tile([128, NL], f32)
                    for dc in range(ND):
                        nc.tensor.matmul(
                            scores,
                            xn_t[:, dc * 128 : (dc + 1) * 128],
                            xT_sb[:, dc, :],
                            start=(dc == 0),
                            stop=(dc == ND - 1),
                        )
                    nc.vector.scalar_tensor_tensor(
                        out=dbuf[:, jl, :],
                        in0=scores,
                        scalar=xnsq_sb[:, j : j + 1],
                        in1=xsq_bc,
                        op0=mybir.AluOpType.add,
                        op1=mybir.AluOpType.add,
                    )
                # group-level software pipeline: PE runs prior group's
                # upsum while this group's activations are still in flight
                if prev is not None:
                    upsum_block(*prev)
                nc.scalar.activation(out=dbuf[:], in_=dbuf[:], func=AF.Sqrt)
                nc.scalar.activation(
                    out=ebuf[:],
                    in_=dbuf[:],
                    func=AF.Exp,
                    scale=-1.0,
                    bias=shift_sb[:, 0:1],
                )
                prev = (g, ebuf)

            upsum_block(*prev)
            nc.vector.tensor_copy(out=out_sb, in_=upsum)
            nc.sync.dma_start(out=out_u_h.ap(), in_=out_sb)
            nc.sync.dma_start(out=out_es_h.ap(), in_=esum)

    nc.compile()
    return nc


def kernel(x, x_n, y, log_T):
    global _CACHED_NC, LAST_RESULT
    from concourse.bass_utils import run_bass_kernel_spmd

    x = np.ascontiguousarray(np.asarray(x, dtype=np.float32))
    x_n = np.ascontiguousarray(np.asarray(x_n, dtype=np.float32))
    y = np.ascontiguousarray(np.asarray(y, dtype=np.float32))

    if _CACHED_NC is None:
        _CACHED_NC = _build_nc()
    nc = _CACHED_NC

    xnT2 = np.ascontiguousarray(
        (-2.0 * x_n)
        .astype(np.float16)
        .reshape(NJ, 128, ND, 128)
        .transpose(0, 3, 2, 1)
        .reshape(NJ, 128, D)
    )
    yt = np.ascontiguousarray(y.reshape(NJ, 128, C).transpose(1, 0, 2).astype(np.float16))
    xnsq = np.ascontiguousarray((x_n * x_n).sum(axis=1).reshape(NJ, 128).T)
    ones = np.ones((1, 128), dtype=np.float32)
    shiftv = np.full((128, 1), SHIFT, dtype=np.float32)

    in_maps = []
    for i in range(NCORES):
        xs = x[i * NL : (i + 1) * NL]
        xT = np.ascontiguousarray(
            xs.astype(np.float16).reshape(NL, ND, 128).transpose(2, 1, 0)
        )
        xsq = np.ascontiguousarray((xs * xs).sum(axis=1)[None, :])
        in_maps.append(
            {
                "xnT2": xnT2,
                "xT": xT,
                "yt": yt,
                "xnsq": xnsq,
                "xsq": xsq,
                "ones": ones,
                "shiftv": shiftv,
            }
        )

    trace = os.environ.get("KERNEL_TRACE") == "1"
    res = run_bass_kernel_spmd(nc, in_maps, list(range(NCORES)), trace=trace)
    LAST_RESULT = res

    out = np.empty((N, C), dtype=np.float32)
    for i in range(NCORES):
        u_t = res.results[i]["out_u"]  # [C, NL]
        es = res.results[i]["out_es"]  # [128, NL]
        denom = es.sum(axis=0, dtype=np.float64)  # [NL]
        out[i * NL : (i + 1) * NL] = (u_t / denom[None, :]).T.astype(np.float32)
    return out


# revision 21
# speedup vs baseline: 1.3304x; 1.3304x over previous
import os
import sys

sys.path.insert(0, "/opt/trn_rl_repo")
import numpy as np

N, M, D, C = 4096, 8192, 1024, 128
NCORES = 8
NL = N // NCORES  # 512 query rows per core
NJ = M // 128  # 64 xn chunks
ND = D // 128  # 8 contraction chunks
G = 16  # xn chunks per activation phase group
NG = NJ // G
CH = 4  # xn chunks per ACT instruction (sub-chunk of a group)
SHIFT = 45.0  # ~mean distance; exp(SHIFT - dist) stays in fp16 range

_CACHED_NC = None
LAST_RESULT = None


def _build_nc():
    import concourse.bacc as bacc
    import concourse.mybir as mybir
    import concourse.tile as tile
    import concourse.bass as bass

    f32 = mybir.dt.float32
    f16 = mybir.dt.float16
    AF = mybir.ActivationFunctionType

    nc = bacc.Bacc(target_bir_lowering=False)
    xnT2_h = nc.declare_dram_parameter("xnT2", [NJ, 128, D], f16, isOutput=False)
    xT_h = nc.declare_dram_parameter("xT", [128, ND, NL], f16, isOutput=False)
    yt_h = nc.declare_dram_parameter("yt", [128, NJ, C], f16, isOutput=False)
    xnsq_h = nc.declare_dram_parameter("xnsq", [128, NJ], f32, isOutput=False)
    xsq_h = nc.declare_dram_parameter("xsq", [1, NL], f32, isOutput=False)
    ones_h = nc.declare_dram_parameter("ones", [1, 128], f32, isOutput=False)
    shift_h = nc.declare_dram_parameter("shiftv", [128, 1], f32, isOutput=False)
    out_u_h = nc.declare_dram_parameter("out_u", [C, NL], f32, isOutput=True)
    out_es_h = nc.declare_dram_parameter("out_es", [128, NL], f32, isOutput=True)

    with tile.TileContext(nc) as tc:
        with (
            tc.tile_pool(name="const", bufs=1) as cpool,
            tc.tile_pool(name="dgrp", bufs=2) as dpool,
            tc.tile_pool(name="egrp", bufs=2) as epool,
            tc.tile_pool(name="stream", bufs=6) as spool,
            tc.tile_pool(name="scps", bufs=2, space=bass.MemorySpace.PSUM) as ppool,
            tc.tile_pool(name="acps", bufs=1, space=bass.MemorySpace.PSUM) as upool,
        ):
            xT_sb = cpool.tile([128, ND, NL], f16)
            y_sb = cpool.tile([128, NJ, C], f16)
            xnsq_sb = cpool.tile([128, NJ], f32)
            xsq_sb = cpool.tile([1, NL], f32)
            ones_sb = cpool.tile([1, 128], f32)
            shift_sb = cpool.tile([128, 1], f32)
            xsq_bc = cpool.tile([128, NL], f32)
            esum = cpool.tile([128, NL], f32)
            out_sb = cpool.tile([C, NL], f32)

            nc.sync.dma_start(out=ones_sb, in_=ones_h[:])
            nc.sync.dma_start(out=xsq_sb, in_=xsq_h[:])
            nc.sync.dma_start(out=xnsq_sb, in_=xnsq_h[:])
            nc.sync.dma_start(out=shift_sb, in_=shift_h[:])
            nc.sync.dma_start(out=xT_sb, in_=xT_h[:])

            # broadcast ||x||^2 row to all 128 partitions via K=1 matmul
            xsq_ps = ppool.tile([128, NL], f32)
            nc.tensor.matmul(xsq_ps, ones_sb, xsq_sb, start=True, stop=True)
            nc.vector.tensor_copy(out=xsq_bc, in_=xsq_ps)

            upsum = upool.tile([C, NL], f32)

            def upsum_block(pg, pebuf):
                for jl in range(G):
                    pj = pg * G + jl
                    nc.tensor.matmul(
                        upsum,
                        y_sb[:, pj, :],
                        pebuf[:, jl, :],
                        start=(pj == 0),
                        stop=(pj == NJ - 1),
                    )
                    if pj == 0:
                        nc.vector.tensor_copy(out=esum, in_=pebuf[:, jl, :])
                    else:
                        nc.vector.tensor_add(out=esum, in0=esum, in1=pebuf[:, jl, :])

            prev = None
            for g in range(NG):
                if g == 1:
                    # deferred so group-0 xn stream wins the DMA queues first
                    nc.sync.dma_start(out=y_sb, in_=yt_h[:])
                dbuf = dpool.tile([128, G, NL], f32)
                ebuf = epool.tile([128, G, NL], f16)
                for jl in range(G):
                    j = g * G + jl
                    xn_t = spool.tile([128, D], f16)
                    nc.sync.dma_start(out=xn_t, in_=xnT2_h[j])
                    scores = ppool.tile([128, NL], f32)
                    for dc in range(ND):
                        nc.tensor.matmul(
                            scores,
                            xn_t[:, dc * 128 : (dc + 1) * 128],
                            xT_sb[:, dc, :],
                            start=(dc == 0),
                            stop=(dc == ND - 1),
                        )
                    nc.vector.scalar_tensor_tensor(
                        out=dbuf[:, jl, :],
                        in0=scores,
                        scalar=xnsq_sb[:, j : j + 1],
                        in1=xsq_bc,
                        op0=mybir.AluOpType.add,
                        op1=mybir.AluOpType.add,
                    )
                # group-level software pipeline: PE runs prior group's
                # upsum while this group's activations are still in flight
                if prev is not None:
                    upsum_block(*prev)
                for c in range(0, G, CH):
                    nc.scalar.activation(
                        out=dbuf[:, c : c + CH, :],
                        in_=dbuf[:, c : c + CH, :],
                        func=AF.Sqrt,
                    )
                for c in range(0, G, CH):
                    nc.scalar.activation(
                        out=ebuf[:, c : c + CH, :],
                        in_=dbuf[:, c : c + CH, :],
                        func=AF.Exp,
                        scale=-1.0,
                        bias=shift_sb[:, 0:1],
                    )
                prev = (g, ebuf)

            upsum_block(*prev)
            nc.vector.tensor_copy(out=out_sb, in_=upsum)
            nc.sync.dma_start(out=out_u_h.ap(), in_=out_sb)
            nc.sync.dma_start(out=out_es_h.ap(), in_=esum)

    nc.compile()
    return nc


def kernel(x, x_n, y, log_T):
    global _CACHED_NC, LAST_RESULT
    from concourse.bass_utils import run_bass_kernel_spmd

    x = np.ascontiguousarray(np.asarray(x, dtype=np.float32))
    x_n = np.ascontiguousarray(np.asarray(x_n, dtype=np.float32))
    y = np.ascontiguousarray(np.asarray(y, dtype=np.float32))

    if _CACHED_NC is None:
        _CACHED_NC = _build_nc()
    nc = _CACHED_NC

    xnT2 = np.ascontiguousarray(
        (-2.0 * x_n)
        .astype(np.float16)
        .reshape(NJ, 128, ND, 128)
        .transpose(0, 3, 2, 1)
        .reshape(NJ, 128, D)
    )
    yt = np.ascontiguousarray(y.reshape(NJ, 128, C).transpose(1, 0, 2).astype(np.float16))
    xnsq = np.ascontiguousarray((x_n * x_n).sum(axis=1).reshape(NJ, 128).T)
    ones = np.ones((1, 128), dtype=np.float32)
    shiftv = np.full((128, 1), SHIFT, dtype=np.float32)

    in_maps = []
    for i in range(NCORES):
        xs = x[i * NL : (i + 1) * NL]
        xT = np.ascontiguousarray(
            xs.astype(np.float16).reshape(NL, ND, 128).transpose(2, 1, 0)
        )
        xsq = np.ascontiguousarray((xs * xs).sum(axis=1)[None, :])
        in_maps.append(
            {
                "xnT2": xnT2,
                "xT": xT,
                "yt": yt,
                "xnsq": xnsq,
                "xsq": xsq,
                "ones": ones,
                "shiftv": shiftv,
            }
        )

    trace = os.environ.get("KERNEL_TRACE") == "1"
    res = run_bass_kernel_spmd(nc, in_maps, list(range(NCORES)), trace=trace)
    LAST_RESULT = res

    out = np.empty((N, C), dtype=np.float32)
    for i in range(NCORES):
        u_t = res.results[i]["out_u"]  # [C, NL]
        es = res.results[i]["out_es"]  # [128, NL]
        denom = es.sum(axis=0, dtype=np.float64)  # [NL]
        out[i * NL : (i + 1) * NL] = (u_t / denom[None, :]).T.astype(np.float32)
    return out
